# revision 6
# baseline (speedup 1.0000x reference)
"""GroupLinear (block-diagonal 64x[64,64] linear) Trainium2 kernel.

Strategy:
  - Host: cast to fp16, transpose x ([8192, 4096] -> per-core [512, 8192]
    channel-major shards; group-parallel: core c owns groups [8c, 8c+8)),
    and pack the 8 diagonal weight blocks per core into 4 block-diagonal
    [128(in),128(out)] lhsT tiles (W^T layout, two groups per tile).
  - Device (per core): for each of the 4 channel-pair blocks, stream
    [128, 512] token chunks of x^T through the PE
    (out[128 out_ch, N tok] = lhsT.T @ rhs, fp16 in, fp32 PSUM accum),
    copy+downcast PSUM->SBUF, DMA out to y^T. All HBM traffic is
    contiguous; no on-device transposes.
  - Host: concatenate per-core y^T shards, transpose back, upcast to f32.

fp16 keeps scale-relative absmax error ~5e-4 on these inputs (f32 device
I/O is available via GL_DTYPE=f32 at ~2x the HBM traffic).
"""

import os
import sys

import numpy as np

for _p in ("/opt/trn_rl_repo", "/root/.axon_site/_ro/trn_rl_repo"):
    if os.path.isdir(_p) and _p not in sys.path:
        sys.path.insert(0, _p)

import concourse.bass as bass  # noqa: E402
import concourse.tile as tile  # noqa: E402
from concourse import bacc, mybir  # noqa: E402
from concourse.bass_utils import run_bass_kernel_spmd  # noqa: E402

N_CORES = 8
N_TOKENS = 8192
IN_CH = 4096
OUT_CH = 4096
GROUP_NUM = 64
SCALE = 64  # in_scale == out_scale == 64
GROUPS_PER_CORE = GROUP_NUM // N_CORES  # 8
CH_PER_CORE = IN_CH // N_CORES  # 512
PAIRS_PER_CORE = GROUPS_PER_CORE // 2  # 4 (two groups per 128-wide PE tile)
MM_N = 512  # one fp32 PSUM bank

LAST_RESULTS = None
_PROGRAMS = {}

_DTYPES = {
    "f16": (mybir.dt.float16, np.float16),
    "f32": (mybir.dt.float32, np.float32),
}


def _build_program(dtype_key: str, tok_chunk: int):
    dt, _ = _DTYPES[dtype_key]
    nc = bacc.Bacc(None, target_bir_lowering=False, debug=False)
    xt = nc.dram_tensor("xt", [CH_PER_CORE, N_TOKENS], dt, kind="ExternalInput")
    wt = nc.dram_tensor(
        "wt", [PAIRS_PER_CORE, 128, 128], dt, kind="ExternalInput"
    )
    yt = nc.dram_tensor("yt", [CH_PER_CORE, N_TOKENS], dt, kind="ExternalOutput")
    xt_ap, wt_ap, yt_ap = xt.ap(), wt.ap(), yt.ap()

    with tile.TileContext(nc) as tc:
        with (
            tc.tile_pool(name="wp", bufs=1) as wp,
            tc.tile_pool(name="xp", bufs=4) as xp,
            tc.tile_pool(name="yp", bufs=4) as yp,
            tc.tile_pool(name="ps", bufs=8, space="PSUM") as psp,
        ):
            w_sb = wp.tile([128, PAIRS_PER_CORE * 128], dt)
            # SWDGE path for the (tiny) weight loads keeps the HWDGE FIFO
            # free so the first big x loads dispatch immediately.
            nc.gpsimd.dma_start(
                w_sb[:].rearrange("k (p m) -> k p m", p=PAIRS_PER_CORE),
                wt_ap.rearrange("p k m -> k p m"),
            )
            for p in range(PAIRS_PER_CORE):
                for t0 in range(0, N_TOKENS, tok_chunk):
                    x_t = xp.tile([128, tok_chunk], dt)
                    nc.sync.dma_start(
                        x_t[:],
                        xt_ap[p * 128 : (p + 1) * 128, t0 : t0 + tok_chunk],
                    )
                    y_t = yp.tile([128, tok_chunk], dt)
                    for s in range(tok_chunk // MM_N):
                        ps = psp.tile([128, MM_N], mybir.dt.float32)
                        nc.tensor.matmul(
                            ps[:],
                            w_sb[:, p * 128 : (p + 1) * 128],
                            x_t[:, s * MM_N : (s + 1) * MM_N],
                            start=True,
                            stop=True,
                        )
                        # Alternate PSUM->SBUF downcasts across DVE and ACT
                        # so neither engine serializes the store path.
                        if s % 2 == 0:
                            nc.vector.tensor_copy(
                                y_t[:, s * MM_N : (s + 1) * MM_N], ps[:]
                            )
                        else:
                            nc.scalar.copy(
                                y_t[:, s * MM_N : (s + 1) * MM_N], ps[:]
                            )
                    nc.sync.dma_start(
                        yt_ap[p * 128 : (p + 1) * 128, t0 : t0 + tok_chunk],
                        y_t[:],
                    )
    nc.compile()
    return nc


def kernel(x: np.ndarray, weight: np.ndarray) -> np.ndarray:
    global LAST_RESULTS
    x = np.asarray(x)
    weight = np.asarray(weight, dtype=np.float32)
    assert x.shape == (N_TOKENS, IN_CH), x.shape
    assert weight.shape == (OUT_CH, IN_CH), weight.shape

    dtype_key = os.environ.get("GL_DTYPE", "f16")
    tok_chunk = int(os.environ.get("GL_TOK_CHUNK", "4096"))
    _, npdt = _DTYPES[dtype_key]

    key = (dtype_key, tok_chunk)
    if key not in _PROGRAMS:
        _PROGRAMS[key] = _build_program(dtype_key, tok_chunk)
    nc = _PROGRAMS[key]

    # Diagonal blocks: blocks[g] = weight[g*64:(g+1)*64, g*64:(g+1)*64]
    wb = weight.reshape(GROUP_NUM, SCALE, GROUP_NUM, SCALE)
    idx = np.arange(GROUP_NUM)
    blocks = wb[idx, :, idx, :]  # [64, out 64, in 64]

    x_c = np.asarray(x, dtype=npdt)
    in_maps = []
    for c in range(N_CORES):
        xt_c = np.ascontiguousarray(
            x_c[:, c * CH_PER_CORE : (c + 1) * CH_PER_CORE].T
        )
        wt_c = np.zeros((PAIRS_PER_CORE, 128, 128), npdt)
        for p in range(PAIRS_PER_CORE):
            g0 = c * GROUPS_PER_CORE + 2 * p
            wt_c[p, 0:SCALE, 0:SCALE] = blocks[g0].T.astype(npdt)  # [in, out]
            wt_c[p, SCALE:128, SCALE:128] = blocks[g0 + 1].T.astype(npdt)
        in_maps.append({"xt": xt_c, "wt": wt_c})

    trace = os.environ.get("GL_TRACE") == "1"
    res = run_bass_kernel_spmd(
        nc, in_maps, core_ids=list(range(N_CORES)), trace=trace
    )
    LAST_RESULTS = res

    yt_full = np.concatenate(
        [r["yt"] for r in res.results], axis=0
    )  # [4096, 8192]
    return np.ascontiguousarray(yt_full.T.astype(np.float32))


if __name__ == "__main__":
    rng = np.random.default_rng(0)
    x = rng.standard_normal((N_TOKENS, IN_CH), dtype=np.float32)
    w = rng.standard_normal((OUT_CH, IN_CH), dtype=np.float32) / 64.0
    y = kernel(x, w)
    print("out", y.shape, y.dtype)
